# revision 10
# baseline (speedup 1.0000x reference)
"""Trainium2 Bass kernel for the req_to_token scatter problem.

for each request pid:
  req_to_token[req_pool_indices[pid], seq_lens[pid] : seq_lens[pid]+L] =
      out_cache_loc[pid*L : (pid+1)*L]            (L = topk * spec_steps = 64)

Returns (new_req_to_token, out_cache_loc, empty, empty) like the reference.

Distribution: the 512 pool rows are block-sharded across 8 NeuronCores
(64 rows per core).  Each core copies its row-shard DRAM->DRAM and then
scatters the 64-element segments belonging to its rows via an indirect
DMA whose per-partition flat offsets (local_row * row_width + seq_len)
are computed on the host from the index inputs.  No cross-core traffic.

Raw Bass (no Tile): all pre-DMAs (staging loads + bulk copy chunks)
increment one semaphore; the gpsimd stream waits for all of them with a
single wait_ge, then issues the indirect scatters.
"""

import os
from contextlib import ExitStack

import numpy as np

import concourse.bass as bass
import concourse.mybir as mybir
from concourse.bass import IndirectOffsetOnAxis
from concourse.bass_utils import run_bass_kernel_spmd

N_CORES = 8
NUM_POOLS = 512
POOL_LEN = 40960
B = 256
L = 64                      # topk * speculative_num_steps
ROWS_PER_CORE = NUM_POOLS // N_CORES    # 64
CAP = B                     # max requests routed to one core (worst case: all)
PART = 128                  # SBUF partitions per indirect DMA
OOB_PAD = 0x0FFFFFFF        # padding index, beyond bounds_check -> skipped

# bulk-copy split: chunks per HWDGE queue (SP + ACT)
CHUNKS_PER_Q = int(os.environ.get("KERNEL_CHUNKS_PER_Q", "1"))
# outer dim of each copy chunk's access pattern (descriptor count)
DESC_PER_CHUNK = int(os.environ.get("KERNEL_DESC_PER_CHUNK", "16"))

_program_cache: dict[tuple, bass.Bass] = {}

# results of the most recent hardware run (BassKernelResults); test harness
# reads .exec_time_ns from here when tracing is enabled
LAST_RESULTS = None


def _build_program(lanes: int) -> bass.Bass:
    """Build the per-core Bass program.

    Everything on the device is int32; an int64 input is viewed as 2
    int32 lanes per element (lanes = itemsize // 4).
    """
    row_w = POOL_LEN * lanes            # int32 lanes per pool row
    total = ROWS_PER_CORE * row_w       # int32 lanes per core shard
    seg = L * lanes                     # int32 lanes per scatter segment
    n_groups = CAP // PART

    nc = bass.Bass()
    src = nc.dram_tensor(
        "src", [ROWS_PER_CORE, row_w], mybir.dt.int32, kind="ExternalInput"
    )
    # per-request scatter record: [offset, seg values...]
    scat = nc.dram_tensor(
        "scat", [CAP, 1 + seg], mybir.dt.int32, kind="ExternalInput"
    )
    dst = nc.dram_tensor(
        "dst", [ROWS_PER_CORE, row_w], mybir.dt.int32, kind="ExternalOutput"
    )

    n_chunks = 2 * CHUNKS_PER_Q
    assert total % n_chunks == 0
    chunk = total // n_chunks
    assert chunk % DESC_PER_CHUNK == 0
    inner = chunk // DESC_PER_CHUNK
    src_flat = src[:, :].flatten()
    dst_flat = dst[:, :].flatten()

    n_pre = 2 * CHUNKS_PER_Q + n_groups

    with ExitStack() as ctx:
        stage = [
            ctx.enter_context(
                nc.sbuf_tensor(f"stage{g}", [PART, 1 + seg], mybir.dt.int32)
            )
            for g in range(n_groups)
        ]
        pre_sem = ctx.enter_context(nc.semaphore("pre_sem"))
        sc_sem = ctx.enter_context(nc.semaphore("sc_sem"))
        block = ctx.enter_context(nc.Block())

        def chunk_ap(flat, c):
            return flat[c * chunk:(c + 1) * chunk].rearrange(
                "(d i) -> d i", d=DESC_PER_CHUNK
            )

        @block.sync
        def _(sync):
            for g in range(n_groups):
                sync.dma_start(
                    out=stage[g][:, :], in_=scat[g * PART:(g + 1) * PART, :]
                ).then_inc(pre_sem, 16)
            for c in range(CHUNKS_PER_Q):
                sync.dma_start(
                    out=chunk_ap(dst_flat, 2 * c), in_=chunk_ap(src_flat, 2 * c)
                ).then_inc(pre_sem, 16)

        @block.scalar
        def _(scalar):
            for c in range(CHUNKS_PER_Q):
                scalar.dma_start(
                    out=chunk_ap(dst_flat, 2 * c + 1),
                    in_=chunk_ap(src_flat, 2 * c + 1),
                ).then_inc(pre_sem, 16)

        @block.gpsimd
        def _(gpsimd):
            gpsimd.wait_ge(pre_sem, 16 * n_pre)
            dst_ind = dst_flat.unsqueeze(1)  # [total, 1]
            for g in range(n_groups):
                gpsimd.indirect_dma_start(
                    out=dst_ind,
                    out_offset=IndirectOffsetOnAxis(ap=stage[g][:, :1], axis=0),
                    in_=stage[g][:, 1:1 + seg],
                    in_offset=None,
                    bounds_check=total - seg,
                    oob_is_err=False,
                ).then_inc(sc_sem, 16)
            gpsimd.wait_ge(sc_sem, 16 * n_groups)

    return nc


def _get_program(lanes: int) -> bass.Bass:
    key = (lanes, CHUNKS_PER_Q, DESC_PER_CHUNK)
    if key not in _program_cache:
        _program_cache[key] = _build_program(lanes)
    return _program_cache[key]


def kernel(
    req_pool_indices,
    req_to_token,
    seq_lens,
    extend_lens,
    num_new_pages_per_topk,
    out_cache_loc,
    last_page_lens_cumsum,
    duplicate_cache_len,
    topk,
    speculative_num_steps,
    page_size,
):
    global LAST_RESULTS

    rpt = np.ascontiguousarray(np.asarray(req_to_token))
    rpi = np.asarray(req_pool_indices)
    seq = np.asarray(seq_lens)
    ocl = np.ascontiguousarray(np.asarray(out_cache_loc))

    assert int(duplicate_cache_len) == 0
    assert int(page_size) == 1 or int(topk) == 1
    assert int(topk) * int(speculative_num_steps) == L
    assert rpt.shape == (NUM_POOLS, POOL_LEN)
    batch = seq.shape[0]
    assert batch <= B

    dtype = rpt.dtype
    itemsize = dtype.itemsize
    assert itemsize in (4, 8)
    lanes = itemsize // 4
    row_w = POOL_LEN * lanes
    seg = L * lanes

    rpt32 = rpt.view(np.int32).reshape(NUM_POOLS, row_w)
    vals32 = ocl.view(np.int32).reshape(batch, seg)

    rpi64 = rpi.astype(np.int64)
    core_of = rpi64 // ROWS_PER_CORE
    start = (rpi64 % ROWS_PER_CORE) * row_w + seq.astype(np.int64) * lanes

    in_maps = []
    for c in range(N_CORES):
        sel = np.nonzero(core_of == c)[0]
        assert len(sel) <= CAP
        scat_arr = np.zeros((CAP, 1 + seg), np.int32)
        scat_arr[:, 0] = OOB_PAD
        scat_arr[: len(sel), 0] = start[sel].astype(np.int32)
        scat_arr[: len(sel), 1:] = vals32[sel]
        in_maps.append(
            {
                "src": np.ascontiguousarray(
                    rpt32[c * ROWS_PER_CORE:(c + 1) * ROWS_PER_CORE]
                ),
                "scat": scat_arr,
            }
        )

    nc = _get_program(lanes)
    trace = os.environ.get("KERNEL_TRACE", "0") == "1"
    LAST_RESULTS = run_bass_kernel_spmd(
        nc, in_maps, core_ids=list(range(N_CORES)), trace=trace
    )

    out32 = np.concatenate([r["dst"] for r in LAST_RESULTS.results])
    new_rpt = out32.view(dtype).reshape(NUM_POOLS, POOL_LEN)
    empty = np.zeros((0,), dtype=ocl.dtype)
    return (new_rpt, ocl, empty, empty)


# revision 11
# speedup vs baseline: 1.1013x; 1.1013x over previous
"""Trainium2 Bass kernel for the req_to_token scatter problem.

for each request pid:
  req_to_token[req_pool_indices[pid], seq_lens[pid] : seq_lens[pid]+L] =
      out_cache_loc[pid*L : (pid+1)*L]            (L = topk * spec_steps = 64)

Returns (new_req_to_token, out_cache_loc, empty, empty) like the reference.

Distribution: the 512 pool rows are block-sharded across 8 NeuronCores
(64 rows per core).  Each core copies its row-shard DRAM->DRAM and then
scatters the 64-element segments belonging to its rows via indirect
DMAs whose per-partition flat offsets (local_row * row_width + seq_len)
are computed on the host from the index inputs.  No cross-core traffic.

Raw Bass (no Tile).  The shard is copied in two row-halves (SP queue and
ACT queue); the scatter for each half is gated only on that half's copy,
so the first half's scatter overlaps the second half's copy.
"""

import os
from contextlib import ExitStack

import numpy as np

import concourse.bass as bass
import concourse.mybir as mybir
from concourse.bass import IndirectOffsetOnAxis
from concourse.bass_utils import run_bass_kernel_spmd

N_CORES = 8
NUM_POOLS = 512
POOL_LEN = 40960
B = 256
L = 64                      # topk * speculative_num_steps
ROWS_PER_CORE = NUM_POOLS // N_CORES    # 64
HALF_ROWS = ROWS_PER_CORE // 2          # 32
PART = 128                  # scatter slots per half (B/2 worst case fits)
OOB_PAD = 0x0FFFFFFF        # padding index, beyond bounds_check -> skipped

# bulk-copy split: chunks per HWDGE queue (SP + ACT)
CHUNKS_PER_Q = int(os.environ.get("KERNEL_CHUNKS_PER_Q", "1"))
# outer dim of each copy chunk's access pattern (descriptor count)
DESC_PER_CHUNK = int(os.environ.get("KERNEL_DESC_PER_CHUNK", "16"))

_program_cache: dict[tuple, bass.Bass] = {}

# results of the most recent hardware run (BassKernelResults); test harness
# reads .exec_time_ns from here when tracing is enabled
LAST_RESULTS = None


def _build_program(lanes: int) -> bass.Bass:
    """Build the per-core Bass program.

    Everything on the device is int32; an int64 input is viewed as 2
    int32 lanes per element (lanes = itemsize // 4).
    """
    row_w = POOL_LEN * lanes            # int32 lanes per pool row
    total = ROWS_PER_CORE * row_w       # int32 lanes per core shard
    half = HALF_ROWS * row_w            # int32 lanes per half shard
    seg = L * lanes                     # int32 lanes per scatter segment

    nc = bass.Bass()
    src = nc.dram_tensor(
        "src", [ROWS_PER_CORE, row_w], mybir.dt.int32, kind="ExternalInput"
    )
    # per-request scatter record: [offset, seg values...]; half h uses
    # rows [h*PART, (h+1)*PART)
    scat = nc.dram_tensor(
        "scat", [2 * PART, 1 + seg], mybir.dt.int32, kind="ExternalInput"
    )
    dst = nc.dram_tensor(
        "dst", [ROWS_PER_CORE, row_w], mybir.dt.int32, kind="ExternalOutput"
    )

    assert half % CHUNKS_PER_Q == 0
    chunk = half // CHUNKS_PER_Q
    assert chunk % DESC_PER_CHUNK == 0
    src_flat = src[:, :].flatten()
    dst_flat = dst[:, :].flatten()

    def chunk_ap(flat, h, c):
        lo = h * half + c * chunk
        return flat[lo:lo + chunk].rearrange("(d i) -> d i", d=DESC_PER_CHUNK)

    with ExitStack() as ctx:
        stage = [
            ctx.enter_context(
                nc.sbuf_tensor(f"stage{h}", [PART, 1 + seg], mybir.dt.int32)
            )
            for h in range(2)
        ]
        load_sem = ctx.enter_context(nc.semaphore("load_sem"))
        copy_sems = [
            ctx.enter_context(nc.semaphore(f"copy_sem{h}")) for h in range(2)
        ]
        sc_sem = ctx.enter_context(nc.semaphore("sc_sem"))
        block = ctx.enter_context(nc.Block())

        @block.sync
        def _(sync):
            for h in range(2):
                sync.dma_start(
                    out=stage[h][:, :], in_=scat[h * PART:(h + 1) * PART, :]
                ).then_inc(load_sem, 16)
            for c in range(CHUNKS_PER_Q):
                sync.dma_start(
                    out=chunk_ap(dst_flat, 0, c), in_=chunk_ap(src_flat, 0, c)
                ).then_inc(copy_sems[0], 16)

        @block.scalar
        def _(scalar):
            for c in range(CHUNKS_PER_Q):
                scalar.dma_start(
                    out=chunk_ap(dst_flat, 1, c), in_=chunk_ap(src_flat, 1, c)
                ).then_inc(copy_sems[1], 16)

        @block.gpsimd
        def _(gpsimd):
            gpsimd.wait_ge(load_sem, 32)
            dst_ind = dst_flat.unsqueeze(1)  # [total, 1]
            for h in range(2):
                gpsimd.wait_ge(copy_sems[h], 16 * CHUNKS_PER_Q)
                gpsimd.indirect_dma_start(
                    out=dst_ind,
                    out_offset=IndirectOffsetOnAxis(ap=stage[h][:, :1], axis=0),
                    in_=stage[h][:, 1:1 + seg],
                    in_offset=None,
                    bounds_check=total - seg,
                    oob_is_err=False,
                ).then_inc(sc_sem, 16)
            gpsimd.wait_ge(sc_sem, 32)

    return nc


def _get_program(lanes: int) -> bass.Bass:
    key = (lanes, CHUNKS_PER_Q, DESC_PER_CHUNK)
    if key not in _program_cache:
        _program_cache[key] = _build_program(lanes)
    return _program_cache[key]


def kernel(
    req_pool_indices,
    req_to_token,
    seq_lens,
    extend_lens,
    num_new_pages_per_topk,
    out_cache_loc,
    last_page_lens_cumsum,
    duplicate_cache_len,
    topk,
    speculative_num_steps,
    page_size,
):
    global LAST_RESULTS

    rpt = np.ascontiguousarray(np.asarray(req_to_token))
    rpi = np.asarray(req_pool_indices)
    seq = np.asarray(seq_lens)
    ocl = np.ascontiguousarray(np.asarray(out_cache_loc))

    assert int(duplicate_cache_len) == 0
    assert int(page_size) == 1 or int(topk) == 1
    assert int(topk) * int(speculative_num_steps) == L
    assert rpt.shape == (NUM_POOLS, POOL_LEN)
    batch = seq.shape[0]
    assert batch <= B

    dtype = rpt.dtype
    itemsize = dtype.itemsize
    assert itemsize in (4, 8)
    lanes = itemsize // 4
    row_w = POOL_LEN * lanes
    seg = L * lanes

    rpt32 = rpt.view(np.int32).reshape(NUM_POOLS, row_w)
    vals32 = ocl.view(np.int32).reshape(batch, seg)

    rpi64 = rpi.astype(np.int64)
    core_of = rpi64 // ROWS_PER_CORE
    local_row = rpi64 % ROWS_PER_CORE
    half_of = local_row // HALF_ROWS
    start = local_row * row_w + seq.astype(np.int64) * lanes

    in_maps = []
    for c in range(N_CORES):
        scat_arr = np.zeros((2 * PART, 1 + seg), np.int32)
        scat_arr[:, 0] = OOB_PAD
        for h in range(2):
            sel = np.nonzero((core_of == c) & (half_of == h))[0]
            assert len(sel) <= PART
            scat_arr[h * PART: h * PART + len(sel), 0] = start[sel].astype(
                np.int32
            )
            scat_arr[h * PART: h * PART + len(sel), 1:] = vals32[sel]
        in_maps.append(
            {
                "src": np.ascontiguousarray(
                    rpt32[c * ROWS_PER_CORE:(c + 1) * ROWS_PER_CORE]
                ),
                "scat": scat_arr,
            }
        )

    nc = _get_program(lanes)
    trace = os.environ.get("KERNEL_TRACE", "0") == "1"
    LAST_RESULTS = run_bass_kernel_spmd(
        nc, in_maps, core_ids=list(range(N_CORES)), trace=trace
    )

    out32 = np.concatenate([r["dst"] for r in LAST_RESULTS.results])
    new_rpt = out32.view(dtype).reshape(NUM_POOLS, POOL_LEN)
    empty = np.zeros((0,), dtype=ocl.dtype)
    return (new_rpt, ocl, empty, empty)
